# revision 22
# baseline (speedup 1.0000x reference)
"""Trainium2 Bass kernel for CrossModalAttention — v5.

Reference (B=1, C=64, N=8192): two cross-attention directions (CT queries
over MRI K/V and vice versa), each an 8192x8192 softmax attention, fused
output projection.

Sharding: each of 8 cores owns 1024 queries for BOTH directions; K/V span
the full sequence.  The host precomputes the Q/K projection composed into
one matrix applied to the query features, the V projection, and the
DoubleRow-interleaved fp8 V^T layout, so the device does ONLY the O(N^2)
work: scores matmul -> exp -> AV matmul.  Softmax normalization and the
output projection run on the host from the returned unnormalized
accumulators (65th row = denominator via a ones-column in V^T).

v5 over v4/v3:
 * HAM warm-up primer (v4): the PE clock-gate (1.2 -> 2.4 GHz) only opens
   after ~3.4us of UNINTERRUPTED matmul activity, which the real stream
   (micro-stalls on exp consumers) never provides; ~5us of dummy
   back-to-back matmuls inside the initial DMA window fire it once, and
   re-throttle needs a ~3.4us fully-idle window that never occurs.
   Measured: every matmul dropped from the cold 427ns/512-col rate to the
   warm 216ns rate (183.5us -> 109.7us).
 * Row-paired scores (v5): the 65th contract row (folded bias) contributes
   a per-QUERY constant that softmax cancels, so the scores contraction is
   really 64-dim.  Two 64-row matmuls for adjacent 128-key chunks run
   CONCURRENTLY in the 128-row PE array (row tiling; tile_position derives
   from base_partition) against a qq operand duplicated on partitions
   64..127 — one 128-partition SBUF stream feeds both.  Scores streaming
   time halves (54.6us -> ~28us).
 * Merged direction loop (v7): one software-pipelined loop over both
   directions' key-groups; dir-0's trailing AV matmuls and copy-out
   overlap dir-1's first score pairs (no engine-idle boundary bubble).
   Steady state is ~1.32us per 256-key group, paced by the PE's
   LDWEIGHTS/drain switching (every LDWEIGHTS row-conflicts with the
   in-flight matmul, so ~460ns/group is exposed; walrus emits one
   LDWEIGHTS per matmul with no dedupe) with the exp engines ~86% busy
   just underneath (ScalarE ~1147ns + DVE ~1226ns per [128,1024] tile,
   both pinned at 1 col/cycle from PSUM).  A 1024-col matmul output
   would halve the switch count but PSUM forbids bank-crossing writes.

Device inner loop per 256-key group jg (g = 0..63 across both dirs):
  4 score MMs (2 concurrent row-tiled pairs, one per 512-query block) ->
  exp per (jg, query-block) tile [128,1024]: ScalarE ACTIVATE Exp for
  block 0, DVE Schraudolph bit-trick (round(s*8*log2e + 55.5) as int8 =
  fp8e4 bits of ~exp(s)) for block 1 ->
  AV fp8 DoubleRow MMs trailing LAG groups so the PE never waits on exp.
Steady state ~1.32us/group.  Pacing is the PSUM-ring x DVE chain, not
PE switching: consecutive score pairs pipeline at 216ns when the ring is
empty (fill phase), and slot reuse needs (pair-end -> exp-start ~520ns +
DVE 1223ns + sem)/1.5 slots ~= 1.25us/group; DVE ~93% / ScalarE ~87%
busy.  Merging exp tiles ([128,2048] per instr, amortizing the ~300ns
init) holds 2 ring slots for the full op and pushes the ring bound to
~1.9us/group — strictly worse with only 6 ring banks + 2 att banks.  Final tweaks: warmup writes an ap slot (not the sp ring, which
would advance the first ring-wrap stall half a group), accumulator copies
lean on ScalarE with the last direction's pair split across engines, and
no SCAL_BOTH rebalancing (a 2-tile ScalarE group stalls the ring ~650ns
at jg+LAG while exp throughput isn't even the wall).  All four tail DMAs
stay on the sync queue: a late gpsimd-queue descriptor adds a ~2.4us
queue-drain to the (otherwise fixed ~10us) framework epilogue.

Accuracy: scores in fp16, exp/V in fp8e4 (~3% quantization); softmax
renormalizes shared scale errors away; HW rel err ~4-6e-3 vs 2e-2 gate.
"""

from contextlib import ExitStack

import numpy as np
import ml_dtypes

import concourse.bass as bass
import concourse.mybir as mybir
import concourse.tile as tile
from concourse import bacc
from concourse.bass_utils import run_bass_kernel_spmd

F32 = mybir.dt.float32
F16 = mybir.dt.float16
I8 = mybir.dt.int8
F8 = mybir.dt.float8e4

C = 64          # channels
N = 8192        # voxels (8*32*32)
NCORES = 8
NQ = N // NCORES      # 1024 queries per core
IH = 512              # query block (PSUM bank width in f32)
NIH = NQ // IH        # 2
W = C + 1             # 65: augmented channel dim (AV output only)
JG = 256              # keys per group (DoubleRow contracts 2x128)
NJG = N // JG         # 32 groups
VP = 80               # V^T bytes per (group, half): 65 padded to 16B align
LAG = 2               # key-groups the AV matmuls trail the score matmuls by

LOG2E = 1.4426950408889634
SCHRAUD_K1 = 8.0 * LOG2E       # fp8e4 has 3 mantissa bits
SCHRAUD_K2 = 55.5              # 7 (exp bias) * 8; +-0.5 tunes HW rounding

# key-groups where ScalarE takes BOTH query blocks' exp.  Keep EMPTY:
# exp throughput is NOT the pacing wall (the PE's ~1316ns/group is, and
# DVE's 1226ns/tile fits under it), while a both-tiles group makes
# ScalarE serialize 2x1147ns and stalls the PSUM ring ~650ns a few
# groups later (measured as PE gaps at jg ~= SCAL_BOTH + LAG).
SCAL_BOTH = ()

# feat2 subtile split (cols, 128 per key-group): small first tile so
# compute starts early
FSPLIT = [256, 768, 1024, 1024, 1024]
FOFF = np.cumsum([0] + FSPLIT).tolist()


def _emit_feat_load(eng, featp, feat_dram, name):
    subs = []
    for s, w in enumerate(FSPLIT):
        t = featp.tile([128, w], F16, tag="feat", name=f"{name}{s}")
        eng.dma_start(t[:], feat_dram[:, FOFF[s] : FOFF[s] + w])
        subs.append(t)
    return subs


def _feat_chunk(fs, jg):
    """AP of feat2 columns [128*jg, 128*jg+128) from the split subtiles."""
    j0 = 128 * jg
    for s, w in enumerate(FSPLIT):
        if FOFF[s] <= j0 < FOFF[s + 1]:
            assert j0 + 128 <= FOFF[s + 1]
            return fs[s][:, j0 - FOFF[s] : j0 - FOFF[s] + 128]
    raise AssertionError(jg)


def _emit_attention(nc, pools, streams, acc):
    """One software-pipelined loop over both directions' 32 key-groups:
    direction 0's trailing AV matmuls interleave with direction 1's first
    score pairs, so the exp engines never idle at the boundary."""
    sp, ap, ep, cp = pools["sp"], pools["ap"], pools["ep"], pools["cp"]

    def emit_av(att, d, jg, i, et):
        vt = streams[d][2]
        nc.tensor.matmul(
            att[:],
            lhsT=vt[:]
            .bitcast(F8)
            .rearrange("p (jg two c) -> p jg two c", jg=NJG, two=2)[
                :, jg, :, :W
            ],
            rhs=et[:].bitcast(F8).rearrange("p (two n) -> p two n", two=2),
            perf_mode=mybir.MatmulPerfMode.DoubleRow,
            start=(jg == 0),
            stop=(jg == NJG - 1),
            skip_group_check=True,
        )
        if jg == NJG - 1:
            # this accumulator is complete: copy + DMA out immediately.
            # i-split across engines/queues so the two tail copies and
            # DMAs of the last direction overlap instead of serializing.
            ot = cp.tile([W, IH], F32, tag="ot", name=f"ot{d}{i}")
            if d == 0 or i == 1:  # keep copies off the busier DVE; the
                nc.scalar.copy(ot[:], att[:])  # two tail copies (d=1)
            else:                              # still overlap engines
                nc.vector.tensor_copy(ot[:], att[:])
            nc.sync.dma_start(
                acc[:, NQ * d + IH * i : NQ * d + IH * (i + 1)], ot[:]
            )

    atts = {}
    pending = []
    for g in range(2 * NJG):
        d, jg = divmod(g, NJG)
        if jg == 0:
            atts[d] = [
                ap.tile([W, IH], F32, tag="att", name=f"att{d}{i}")
                for i in range(NIH)
            ]
        fs, qq = streams[d][0], streams[d][1]
        fc = _feat_chunk(fs, jg)
        # two concurrent row-tiled 64-contract matmuls per query block:
        # keys h0 on PE rows 0-63 (from feat2/qq partitions 0-63), keys h1
        # on rows 64-127; tile_position auto-derives from base_partition.
        # A(i0) B(i0) B(i1) A(i1) order keeps consecutive matmuls on the
        # SAME stationary (dedupe/pipelining) while pairs stay concurrent.
        pss = [
            sp.tile([128, 2 * IH], F32, tag="ps", name=f"ps{d}{jg}i{i}")
            for i in range(NIH)
        ]
        qbs = [qq[:, IH * i : IH * (i + 1)] for i in range(NIH)]
        nc.tensor.matmul(pss[0][:, :IH], lhsT=fc[0:64, :], rhs=qbs[0][0:64, :],
                         start=True, stop=True)
        nc.tensor.matmul(pss[0][:, IH:], lhsT=fc[64:128, :], rhs=qbs[0][64:128, :],
                         start=True, stop=True)
        nc.tensor.matmul(pss[1][:, IH:], lhsT=fc[64:128, :], rhs=qbs[1][64:128, :],
                         start=True, stop=True)
        nc.tensor.matmul(pss[1][:, :IH], lhsT=fc[0:64, :], rhs=qbs[1][0:64, :],
                         start=True, stop=True)
        for i in range(NIH):
            ps = pss[i]
            et = ep.tile([128, 2 * IH], I8, tag="exp", name=f"et{d}{jg}i{i}")
            # NOTE: splitting one tile's exp across both engines does NOT
            # halve latency — Tile serializes cross-engine writes to the
            # same tile even for disjoint regions (measured: DVE half
            # starts exactly at ScalarE half's end)
            if i == 0 or jg in SCAL_BOTH:
                nc.scalar.activation(
                    et[:].bitcast(F8), ps[:], mybir.ActivationFunctionType.Exp
                )
            else:
                nc.vector.tensor_scalar(
                    et[:],
                    ps[:],
                    SCHRAUD_K1,
                    SCHRAUD_K2,
                    mybir.AluOpType.mult,
                    mybir.AluOpType.add,
                )
            pending.append((atts[d][i], d, jg, i, et))
            # pop BOTH trailing AVs right after the i0 exp so the unit
            # order is [pair(i0), AV, AV, pair(i1)]: the binding PSUM-ring
            # chain is the i0 slot (Scalar-consumed, reused by the i1-pair
            # 1.5 groups later); pushing the i1-pair later in the group
            # buys that chain ~300ns of runway (p>=1302 -> ~engine wall)
            thresh = 2 * LAG - 1 if i == 0 else 2 * LAG
            while len(pending) > thresh:
                emit_av(*pending.pop(0))
    for args in pending:
        emit_av(*args)


def _build_program(ctx, tc, feat0, feat1, qq0, qq1, vt0, vt1, acc):
    nc = tc.nc
    featp = ctx.enter_context(tc.tile_pool(name="feat", bufs=2 * len(FSPLIT)))
    pools = {
        "qp": ctx.enter_context(tc.tile_pool(name="qp", bufs=2)),
        "vp": ctx.enter_context(tc.tile_pool(name="vp", bufs=2)),
        "ep": ctx.enter_context(tc.tile_pool(name="ep", bufs=9)),
        "cp": ctx.enter_context(tc.tile_pool(name="cp", bufs=2)),
        "wp": ctx.enter_context(tc.tile_pool(name="wp", bufs=1)),
        "sp": ctx.enter_context(tc.tile_pool(name="spsum", bufs=3, space="PSUM")),
        "ap": ctx.enter_context(tc.tile_pool(name="apsum", bufs=2, space="PSUM")),
    }

    # HAM warm-up primer (see module docstring)
    # 12 dummy matmuls fill the initial DMA window (~6.7-11.5us) with
    # UNINTERRUPTED PE activity so HAM un-throttles before the real
    # stream starts; fewer leaves a DMA gap that resets the busy window
    # (measured: 4 warmups -> cold until 18us, net loss).
    wsrc = pools["wp"].tile([128, 640], F16, tag="w", name="wsrc")
    nc.gpsimd.memset(wsrc[:], 0.0)
    # warmup output goes to an att (ap) slot, NOT the sp ring: stealing an
    # sp slot makes the first ring wrap (and its pipeline-fill stall)
    # arrive half a key-group earlier
    wps = pools["ap"].tile([W, IH], F32, tag="att", name="warm")
    for _ in range(12):
        nc.tensor.matmul(
            wps[:], lhsT=wsrc[:, :W], rhs=wsrc[:, 128:128 + IH],
            start=True, stop=True,
        )

    # sync queue: dir-0 tensors in first-use order; gpsimd queue: dir-1
    # prefetch in parallel (it is otherwise idle)
    qq0_sb = pools["qp"].tile([128, NQ], F16, tag="qq", name="qq0")
    nc.sync.dma_start(qq0_sb[:, :IH], qq0[:, :IH])
    vt0_sb = pools["vp"].tile([128, NJG * 2 * VP], I8, tag="vt", name="vt0")
    # first feature subtile rides the gpsimd queue: its transfer and
    # completion semaphore overlap qq0's on the sync queue
    t0 = featp.tile([128, FSPLIT[0]], F16, tag="feat", name="f0_0")
    nc.gpsimd.dma_start(t0[:], feat0[:, : FSPLIT[0]])
    fs0 = [t0]
    for s, w in enumerate(FSPLIT):
        if s == 0:
            continue
        t = featp.tile([128, w], F16, tag="feat", name=f"f0_{s}")
        nc.sync.dma_start(t[:], feat0[:, FOFF[s] : FOFF[s] + w])
        fs0.append(t)
        if s == 1:
            nc.sync.dma_start(qq0_sb[:, IH:], qq0[:, IH:])
        if s <= 4:  # interleave V^T quarters so the first AV isn't starved
            q = NJG * 2 * VP // 4
            nc.sync.dma_start(
                vt0_sb[:, q * (s - 1) : q * s], vt0[:, q * (s - 1) : q * s]
            )

    qq1_sb = pools["qp"].tile([128, NQ], F16, tag="qq", name="qq1")
    nc.sync.dma_start(qq1_sb[:], qq1[:])
    fs1 = _emit_feat_load(nc.sync, featp, feat1, "f1_")
    vt1_sb = pools["vp"].tile([128, NJG * 2 * VP], I8, tag="vt", name="vt1")
    nc.sync.dma_start(vt1_sb[:], vt1[:])

    _emit_attention(
        nc, pools, [(fs0, qq0_sb, vt0_sb), (fs1, qq1_sb, vt1_sb)], acc
    )


def build_bass():
    nc = bacc.Bacc("TRN2", target_bir_lowering=False, debug=False)
    feat0 = nc.dram_tensor("feat0", [128, NJG * 128], F16, kind="ExternalInput").ap()
    feat1 = nc.dram_tensor("feat1", [128, NJG * 128], F16, kind="ExternalInput").ap()
    qq0 = nc.dram_tensor("qq0", [128, NQ], F16, kind="ExternalInput").ap()
    qq1 = nc.dram_tensor("qq1", [128, NQ], F16, kind="ExternalInput").ap()
    vt0 = nc.dram_tensor("vt0", [128, NJG * 2 * VP], I8, kind="ExternalInput").ap()
    vt1 = nc.dram_tensor("vt1", [128, NJG * 2 * VP], I8, kind="ExternalInput").ap()
    acc = nc.dram_tensor("acc", [W, 2 * NQ], F32, kind="ExternalOutput").ap()

    with tile.TileContext(nc) as tc, ExitStack() as ctx:
        _build_program(ctx, tc, feat0, feat1, qq0, qq1, vt0, vt1, acc)
    nc.compile()
    return nc


def _aug(w, b):
    # (out,in) weight + (out,) bias -> [w.T; b] of shape (in+1, out)
    return np.concatenate(
        [np.asarray(w, np.float32).T, np.asarray(b, np.float32)[None, :]], axis=0
    )


def _wkb(w, b):
    # (64, 65): [wk | bk] -- K projection folded onto the query side
    return np.concatenate(
        [np.asarray(w, np.float32), np.asarray(b, np.float32)[:, None]], axis=1
    )


def prepare_inputs(inputs):
    scale = np.float32(1.0 / np.sqrt(C))
    ct = np.asarray(inputs["ct_features"], np.float32).reshape(C, N)
    mri = np.asarray(inputs["mri_features"], np.float32).reshape(C, N)
    ones = np.ones((1, N), np.float32)
    ct_aug = np.concatenate([ct, ones], axis=0)
    mri_aug = np.concatenate([mri, ones], axis=0)

    # scores s[i,j] = (Wq q_i + bq) . (Wk k_j + bk) * scale
    #              = qq[:, i] . feat_aug[:, j],  qq = (Wq_aug @ [Wk|bk])^T @ q_aug
    # The bk column (index 64 of [Wk|bk]) pairs with the ones-row of
    # feat_aug and contributes a per-QUERY constant -> softmax-invariant
    # -> dropped, so the device contraction is 64-dim.
    wqq0 = _aug(
        np.asarray(inputs["wq_ct"]) * scale, np.asarray(inputs["bq_ct"]) * scale
    ) @ _wkb(inputs["wk_mri"], inputs["bk_mri"])
    wqq1 = _aug(
        np.asarray(inputs["wq_mri"]) * scale, np.asarray(inputs["bq_mri"]) * scale
    ) @ _wkb(inputs["wk_ct"], inputs["bk_ct"])
    qq0_full = (wqq0[:, :C].T @ ct_aug).astype(np.float16)   # (64, N)
    qq1_full = (wqq1[:, :C].T @ mri_aug).astype(np.float16)

    def feat2_pack(feat):
        # (64, N) channels -> (128, NJG*128): key-group jg's columns hold
        # keys jg*256+[0,128) channels on partitions 0-63 and keys
        # jg*256+128+[0,128) on partitions 64-127 (row-paired layout)
        x = np.asarray(feat, np.float32).reshape(C, NJG, 2, 128)
        x = x.transpose(2, 0, 1, 3).reshape(128, NJG * 128)
        return np.ascontiguousarray(x).astype(np.float16)

    def vt_pack(w, b, feat_aug):
        # v_aug (65, N): V projection + ones row (softmax denominator), then
        # DoubleRow layout (128, NJG*2*VP): [p, jg*160 + h*80 + c] =
        # v_aug[c, 256*jg + 128*h + p], as fp8e4 bits viewed int8
        v = _aug(w, b).T @ feat_aug                     # (64, N)
        v_aug = np.concatenate([v, np.ones((1, N), np.float32)], axis=0)
        x = v_aug.reshape(W, NJG, 2, 128)               # (c, jg, h, p)
        x = x.transpose(3, 1, 2, 0)                     # (p, jg, h, c)
        xp = np.zeros((128, NJG, 2, VP), np.float32)    # pad c to 16B align
        xp[:, :, :, :W] = x
        xp = np.ascontiguousarray(xp.reshape(128, NJG * 2 * VP))
        return xp.astype(ml_dtypes.float8_e4m3).view(np.int8)

    vt0 = vt_pack(inputs["wv_mri"], inputs["bv_mri"], mri_aug)
    vt1 = vt_pack(inputs["wv_ct"], inputs["bv_ct"], ct_aug)
    feat0 = feat2_pack(mri)
    feat1 = feat2_pack(ct)

    in_maps = []
    for i in range(NCORES):
        sl = slice(NQ * i, NQ * (i + 1))
        qq0_c = qq0_full[:, sl]
        qq1_c = qq1_full[:, sl]
        in_maps.append(
            {
                "feat0": feat0,
                "feat1": feat1,
                # duplicated on partitions 64-127: one 128-partition SBUF
                # column stream feeds both concurrent row-tiled matmuls
                "qq0": np.ascontiguousarray(np.concatenate([qq0_c, qq0_c], axis=0)),
                "qq1": np.ascontiguousarray(np.concatenate([qq1_c, qq1_c], axis=0)),
                "vt0": vt0,
                "vt1": vt1,
            }
        )
    return in_maps


def assemble_output(results, inputs):
    # acc rows 0..63 = sum_j exp * v (unnormalized), row 64 = sum_j exp
    acc = np.concatenate(
        [results[i]["acc"].reshape(W, 2, NQ) for i in range(NCORES)], axis=2
    )  # (65, 2, N)
    att = acc[:C] / acc[C:C + 1]  # (64, 2, N)
    fused = np.concatenate([att[:, 0], att[:, 1]], axis=0)  # (128, N)
    wo = np.asarray(inputs["wo"], np.float32)
    bo = np.asarray(inputs["bo"], np.float32)
    out = wo @ fused + bo[:, None]
    return out.reshape(1, C, 8, 32, 32).astype(np.float32)


_NC_CACHE = None


def _get_nc():
    global _NC_CACHE
    if _NC_CACHE is None:
        _NC_CACHE = build_bass()
    return _NC_CACHE


def kernel(**inputs):
    nc = _get_nc()
    in_maps = prepare_inputs(inputs)
    res = run_bass_kernel_spmd(nc, in_maps, list(range(NCORES)))
    return assemble_output(res.results, inputs)


if __name__ == "__main__":
    nc = build_bass()
    print("built OK")
